# revision 6
# baseline (speedup 1.0000x reference)
"""Bilateral filter (3x3, sigma=0.8) Trainium2 Bass kernel — v2.

Sharding: fully data-parallel over the fused batch B*V = 8 -> one
(C=3,H=512,W=512) image per NeuronCore, 8 cores.

Per-core layout: H=512 rows split 4 rows/partition over 128 partitions,
row stride 520 (2 left pad + 512 + 6 right pad) so every 3x3 tap is a
constant flat offset.

Math (same factorization as v1, validated vs reference ~1.2e-3):
  out = num / den    (eps dropped)
  per pair e in {(0,1),(1,0),(1,1),(1,-1)}:
    E_k = DErf(sqrt(S) * (plane_k[+e] - plane_k)), planes (d,c0,c1,c2)
    F_e = E_d * (E_c0 + E_c1 + E_c2)
    taps: A = F_e * X[+e] accumulated @0,  B = F_e * X accumulated @-e
  with X = (m, m*c0, m*c1, m*c2) giving den and the 3 numerators from
  the same product tensors. Center taps are matmuls directly on X.

v2 changes vs v1:
  - all four planes packed per op: 1 sub + 1 DErf + products per pair
  - odd-column-shifted replicas of the input planes loaded via DMA so
    every DVE tensor_tensor op keeps 4B alignment (2x perf mode)
  - er=1 pair fields computed on 4 rows only; the row -1 duplicate is
    filled from partition p-1 row 3 by a tiny SBUF->SBUF DMA
  - center taps via a third scaled identity (no ncc/m3w0 vector ops)
  - tap matmuls grouped by weight matrix (W1, W2, center)
  - two row-halves so den+3*num accumulators exactly fill 8 PSUM banks
"""

import math
import numpy as np
import sys

if "/opt/trn_rl_repo" not in sys.path:
    sys.path.insert(0, "/opt/trn_rl_repo")

import concourse.bass as bass
import concourse.tile as tile
from concourse import mybir
from concourse.bass_utils import run_bass_kernel_spmd

# ---- problem constants (hardcoded per spec) ----
B, V, C, H, W = 2, 4, 3, 512, 512
N_CORES = 8
KS = 3
SIG = 0.3 * ((KS - 1) * 0.5 - 1) + 0.8           # 0.8
S = 1.0 / (2.0 * SIG * SIG)                       # 0.78125
SQS = math.sqrt(S)
PHI2 = 4.0 / math.pi

# spatial gaussian, normalized
_xs = np.arange(KS, dtype=np.float64)
_gx, _gy = np.meshgrid(_xs, _xs, indexing="xy")
_w = np.exp(-(((_gx - 1) ** 2 + (_gy - 1) ** 2)) * S)
_w = _w / _w.sum()
W0 = float(_w[1, 1])   # center
W1 = float(_w[0, 1])   # edge-adjacent
W2 = float(_w[0, 0])   # diagonal

# layout constants
R = 4                  # data rows per partition
W2C = 520              # row stride (2 left pad + 512 data + 6 right pad)
COL0 = 2               # first data col
PROD = 4 * W2C         # 2080: 4-row field length

# (er, ec, weight-index) — weight 0 = W1, 1 = W2
ES = [(0, 1, 0), (1, 0, 0), (1, 1, 1), (1, -1, 1)]

F16 = mybir.dt.float16
F32 = mybir.dt.float32
AF = mybir.ActivationFunctionType


# ---- walrus single-wait workaround ----------------------------------------
# This container's walrus accepts only ONE sync_info.on_wait per instruction;
# Tile emits multi-wait instructions. Hoist all but the last wait onto
# injected single-wait instructions just before the original (NoOp for
# compute engines; a dummy 4-byte DMACopy on the same HWDGE queue for DMAs).
import orjson as _orjson

_SCRATCH = "wsplit_scratch"


def _mk_nop(name, engine, wait):
    return {"name": name, "engine": engine, "ins": [], "outs": [],
            "opcode": "NoOp",
            "sync_info": {"on_wait": [wait], "on_update": []}}


def _mk_dummy_dma(name, proto, wait):
    ap = {"ap": [[1, 1], [1, 1]], "dtype": "float32", "kind": "physical_ap",
          "memref": _SCRATCH, "memsetref": _SCRATCH + "_set", "offset": 0}
    d = {"name": name, "engine": proto["engine"], "opcode": "DMACopy",
         "mode": "Copy", "cce_op": "bypass", "single_packet": False,
         "ins": [ap], "outs": [dict(ap, offset=2)],
         "sync_info": {"on_wait": [wait], "on_update": []}}
    for k in ("queue", "oob_is_err"):
        if k in proto:
            d[k] = proto[k]
    return d


def _split_multiwaits(bir_bytes):
    m = _orjson.loads(bir_bytes)
    for f in m.get("functions", []):
        for bb in f.get("blocks", []):
            out = []
            for ins in bb.get("instructions", []):
                si = ins.get("sync_info")
                waits = (si or {}).get("on_wait") or []
                if len(waits) > 1:
                    for k, w in enumerate(waits[:-1]):
                        nm = f"{ins['name']}-wsplit{k}"
                        out.append(_mk_nop(nm, ins["engine"], w))
                    si["on_wait"] = [waits[-1]]
                out.append(ins)
            bb["instructions"] = out
    return _orjson.dumps(m)


_BUILD_CACHE = {}


def _build_nc():
    nc = bass.Bass()
    xd_in = nc.declare_dram_parameter("xd", [4, 128, 5, W], F16, isOutput=False)
    xx_in = nc.declare_dram_parameter("xx", [4, 128, 6, W], F16, isOutput=False)
    idw_in = nc.declare_dram_parameter("identw", [3, 128, 128], F16, isOutput=False)
    o_out = nc.declare_dram_parameter("out", [C, H, W], F16, isOutput=True)
    nc.dram_tensor(_SCRATCH, [4], F32)

    with tile.TileContext(nc) as tc:
        _emit(nc, tc, xd_in, xx_in, idw_in, o_out)

    orig_to_json = nc.to_json_bytes
    nc.to_json_bytes = lambda: _split_multiwaits(orig_to_json())
    return nc


def _emit(nc, tc, xd_in, xx_in, idw_in, o_out):
    from contextlib import ExitStack
    ctx = ExitStack()
    with ctx:
        persist = ctx.enter_context(tc.tile_pool(name="persist", bufs=1))
        ef_p = ctx.enter_context(tc.tile_pool(name="ef", bufs=2))
        g_p = ctx.enter_context(tc.tile_pool(name="g", bufs=1))
        t_p = ctx.enter_context(tc.tile_pool(name="t", bufs=2))
        ev_p = ctx.enter_context(tc.tile_pool(name="ev", bufs=1))
        psum_p = ctx.enter_context(
            tc.tile_pool(name="psum", bufs=1, space=bass.MemorySpace.PSUM)
        )

        # ---- persistent input planes: [even/odd, plane, row-slot, col] ----
        Dte = persist.tile([128, 2, 4, 5, W2C], F16, tag="Dte", name="Dte")
        Xte = persist.tile([128, 2, 4, 6, W2C], F16, tag="Xte", name="Xte")
        identw = persist.tile([128, 3, 128], F16, tag="identw", name="identw")
        Ftl = persist.tile([128, 4, 5, W2C], F16, tag="Ftl", name="Ftl")

        # zero the pad columns (and the F halo slots, incl partition 0)
        nc.vector.memset(Dte[:, 0, :, :, 0:COL0], 0.0)
        nc.vector.memset(Dte[:, 0, :, :, COL0 + W:W2C], 0.0)
        nc.vector.memset(Dte[:, 1, :, :, 0:1], 0.0)
        nc.vector.memset(Dte[:, 1, :, :, 1 + W:W2C], 0.0)
        nc.vector.memset(Xte[:, 0, :, :, 0:COL0], 0.0)
        nc.vector.memset(Xte[:, 0, :, :, COL0 + W:W2C], 0.0)
        nc.vector.memset(Xte[:, 1, :, :, 0:1], 0.0)
        nc.vector.memset(Xte[:, 1, :, :, 1 + W:W2C], 0.0)
        nc.vector.memset(Ftl[:, :, 0:1, :], 0.0)

        # ---- loads: even copies + odd (left-shifted-by-1) replicas ----
        for k in range(4):
            nc.sync.dma_start(Dte[:, 0, k, :, COL0:COL0 + W], xd_in[k])
            nc.scalar.dma_start(Dte[:, 1, k, :, 1:1 + W], xd_in[k])
            nc.sync.dma_start(Xte[:, 0, k, :, COL0:COL0 + W], xx_in[k])
            nc.scalar.dma_start(Xte[:, 1, k, :, 1:1 + W], xx_in[k])
        nc.sync.dma_start(identw[:], idw_in.rearrange("j p c -> p j c"))

        # flat per-copy views [128, plane, flat]
        Dfe = Dte[:, 0].rearrange("p a b c -> p a (b c)")
        Dfo = Dte[:, 1].rearrange("p a b c -> p a (b c)")
        Xfe = Xte[:, 0].rearrange("p a b c -> p a (b c)")
        Xfo = Xte[:, 1].rearrange("p a b c -> p a (b c)")

        # ---- phase A: pair fields ----
        for i, (er, ec, wi) in enumerate(ES):
            # sub operand offset in the flat (slot,col) grid; odd copy
            # absorbs the odd column component
            odd = (ec % 2) != 0
            off = er * W2C + (ec - 1 if odd else ec)
            src = Dfo if odd else Dfe
            Ez = ef_p.tile([128, 4, PROD], F16, tag="Ez", name="Ez")
            nc.vector.tensor_sub(
                Ez[:], src[:, :, off:off + PROD], Dfe[:, :, 0:PROD]
            )
            nc.scalar.activation(Ez[:], Ez[:], AF.Derivative_Erf, scale=SQS)
            G = g_p.tile([128, PROD], F16, tag="G", name="G")
            nc.vector.tensor_add(G[:], Ez[:, 1], Ez[:, 2])
            nc.vector.tensor_add(G[:], G[:], Ez[:, 3])
            Fout = Ftl[:, i, 1:5, :].rearrange("p a b -> p (a b)")
            nc.vector.tensor_mul(Fout, Ez[:, 0], G[:])
            if er == 1:
                # fill field halo row -1 from partition p-1's row 3
                nc.sync.dma_start(
                    Ftl[1:128, i, 0:1, :], Ftl[0:127, i, 4:5, :]
                )

        # ---- per-half products + tap accumulation + evac ----
        for h0 in (0, 2):
            acc = [
                psum_p.tile([128, 2, W], F32, tag=f"acc{pl}", name=f"acc{pl}")
                for pl in range(4)
            ]
            TAs, TBs = [], []
            for i, (er, ec, wi) in enumerate(ES):
                odd = (ec % 2) != 0
                # A: F rows h0..h0+1 times X[+e]
                fa = Ftl[:, i, h0 + 1:h0 + 3, :].unsqueeze(1)
                fa = fa.broadcast_to([128, 4, 2, W2C])
                xoff = (h0 + er + 1) * W2C + (ec - 1 if odd else ec)
                xs = Xfo if odd else Xfe
                xa = xs[:, :, xoff:xoff + 2 * W2C].rearrange(
                    "p a (r c) -> p a r c", c=W2C
                )
                TA = t_p.tile([128, 4, 2, W2C], F16, tag="TA", name="TA")
                nc.vector.tensor_mul(TA[:], fa, xa)
                # B: F rows h0-er..h0+1-er times X same rows
                fb = Ftl[:, i, h0 + 1 - er:h0 + 3 - er, :].unsqueeze(1)
                fb = fb.broadcast_to([128, 4, 2, W2C])
                xb = Xte[:, 0, :, h0 + 1 - er:h0 + 3 - er, :]
                TB = t_p.tile([128, 4, 2, W2C], F16, tag="TB", name="TB")
                nc.vector.tensor_mul(TB[:], fb, xb)
                TAs.append(TA)
                TBs.append(TB)

            first = True
            for wi, pairs in ((0, (0, 1)), (1, (2, 3))):
                for i in pairs:
                    ec = ES[i][1]
                    cb = COL0 - ec
                    for pl in range(4):
                        for r in range(2):
                            nc.tensor.matmul(
                                acc[pl][:, r, :], identw[:, wi],
                                TAs[i][:, pl, r, COL0:COL0 + W],
                                start=first, stop=False,
                            )
                            nc.tensor.matmul(
                                acc[pl][:, r, :], identw[:, wi],
                                TBs[i][:, pl, r, cb:cb + W],
                                start=False, stop=False,
                            )
                    first = False
            for pl in range(4):
                for r in range(2):
                    nc.tensor.matmul(
                        acc[pl][:, r, :], identw[:, 2],
                        Xte[:, 0, pl, h0 + 1 + r, COL0:COL0 + W],
                        start=False, stop=True,
                    )

            # evac: r = 1/den via ln+exp; out_c = num_c * r
            ldn = ev_p.tile([128, 2, W], F32, tag="ldn", name="ldn")
            nc.scalar.activation(ldn[:], acc[0][:], AF.Ln)
            r16 = ev_p.tile([128, 2, W], F16, tag="r16", name="r16")
            nc.scalar.activation(r16[:], ldn[:], AF.Exp, scale=-1.0)
            for ci in range(C):
                n16 = ev_p.tile([128, 2, W], F16, tag="n16", name="n16")
                nc.scalar.activation(n16[:], acc[1 + ci][:], AF.Copy)
                o16 = ev_p.tile([128, 2, W], F16, tag="o16", name="o16")
                nc.vector.tensor_mul(o16[:], n16[:], r16[:])
                eng = nc.sync if ci % 2 == 0 else nc.scalar
                eng.dma_start(
                    o_out[ci].rearrange("(p r) w -> p r w", r=R)[:, h0:h0 + 2, :],
                    o16[:],
                )


def _get_nc():
    if "nc" not in _BUILD_CACHE:
        _BUILD_CACHE["nc"] = _build_nc()
    return _BUILD_CACHE["nc"]


def _host_planes(d, c, m):
    """xd [N,4,128,5,512] rows 4p..4p+4; xx [N,4,128,6,512] rows 4p-1..4p+4;
    both fp16 with zero halos."""
    from numpy.lib.stride_tricks import as_strided
    N = N_CORES
    stackD = np.zeros((N, 4, H + 4, W), np.float16)
    stackX = np.zeros((N, 4, H + 5, W), np.float16)
    for i in range(N):
        mi = m[i]
        planesD = (d[i], c[i, 0], c[i, 1], c[i, 2])
        planesX = (mi, mi * c[i, 0], mi * c[i, 1], mi * c[i, 2])
        for k in range(4):
            stackD[i, k, 0:H] = planesD[k]
            stackX[i, k, 1:H + 1] = planesX[k]
    sD = stackD.strides
    xd = as_strided(stackD, shape=(N, 4, 128, 5, W),
                    strides=(sD[0], sD[1], 4 * sD[2], sD[2], sD[3]))
    sX = stackX.strides
    xx = as_strided(stackX, shape=(N, 4, 128, 6, W),
                    strides=(sX[0], sX[1], 4 * sX[2], sX[2], sX[3]))
    return np.ascontiguousarray(xd), np.ascontiguousarray(xx)


def _run(depth, color, mask, trace=False, **kw):
    nc = _get_nc()
    d = np.asarray(depth, dtype=np.float32).reshape(N_CORES, H, W)
    c = np.asarray(color, dtype=np.float32).reshape(N_CORES, C, H, W)
    m = np.asarray(mask, dtype=np.float32).reshape(N_CORES, H, W)
    xd, xx = _host_planes(d, c, m)
    eye = np.eye(128)
    identw = np.stack(
        [eye * W1, eye * W2, eye * (3.0 * W0 * PHI2)]
    ).astype(np.float16)
    in_maps = [
        {"xd": xd[i], "xx": xx[i], "identw": identw} for i in range(N_CORES)
    ]
    res = run_bass_kernel_spmd(
        nc, in_maps, list(range(N_CORES)), trace=trace, **kw
    )
    out = np.stack([np.asarray(res.results[i]["out"]) for i in range(N_CORES)])
    return out.reshape(B, V, C, H, W).astype(np.float32), res


def kernel(depth, color, mask):
    out, _ = _run(depth, color, mask, trace=False)
    return out
